# revision 73
# baseline (speedup 1.0000x reference)
"""MultiHeadSelfAttentionWithLagBias on 8 TRN2 NeuronCores.

Sharding: tensor-parallel over heads — 16 heads / 8 cores = 2 heads per
core. Each core computes QKV projections for its head slice (full x),
attention with the lag bias for its 2 heads over both batch elements,
and a partial output projection (its 128 rows of wo). Host sums the 8
partials and adds bo.

Device layout (per core):
  xT      (1024, 4096)  x transposed, tok = b*2048 + s, bf16
  QT/KT   (128, 4096)   q^T/k^T, partitions = [h0 dk(64) | h1 dk(64)]
  V       (128, 32, 130) per 128-tok chunk: [V_h0(64) | 1 | V_h1(64) | 1]
  scores  computed transposed: S^T (k on partitions, q on free) so the
          softmax denominator falls out of the PV matmul via the ones
          column, and O^T is produced in exactly the layout the output
          projection needs as its stationary operand.
  bias    streamed as EB_h = exp(bias_h) (2048, 2048) bf16, applied as
          pe = exp(scores) * EB on DVE (all-SBUF bf16 runs at the DVE
          2x/4x rate, unlike a PSUM f32 bias add).
  recip   softmax denominators gathered to a [128, 64] tile via an
          SBUF->SBUF DMA transpose so one wide RECIPROCAL covers all
          8192 of them (a [1, N] reciprocal is single-lane and slow).

Everything through the PE array is bf16 (power: sustained f32r work
trips the HW PE utilization throttle to 50%; bf16 also halves DMA).
"""

import ml_dtypes
import numpy as np
from contextlib import ExitStack

import concourse.bass as bass
import concourse.bacc as bacc
import concourse.mybir as mybir
import concourse.tile as tile
from concourse.bass_utils import run_bass_kernel_spmd
from concourse.masks import make_identity

F32 = mybir.dt.float32
BF16 = mybir.dt.bfloat16
AF = mybir.ActivationFunctionType

N_CORES = 8
B, S, D = 2, 2048, 1024
H, DK = 16, 64
TOK = B * S              # 4096
NQ = 512                 # q-chunk (matmul free dim)
NQC = S // NQ            # 4 q-chunks per batch
NJ = S // 128            # 16 k-chunks per batch
DCH = D // 128           # 8 contraction chunks

# Set by test.py for profiling; harness leaves these untouched.
TRACE = False
TRACE_DIR = None

_CACHED_NC = None


def _body(ctx: ExitStack, tc, aps):
    nc = tc.nc
    xT, wq, wk, wv, bq, bk, bv, wo, B0, B1, out = (
        aps["xT"], aps["wq"], aps["wk"], aps["wv"], aps["bq"], aps["bk"],
        aps["bv"], aps["wo"], aps["B0"], aps["B1"], aps["out"])
    Bh = [B0, B1]

    const = ctx.enter_context(tc.tile_pool(name="const", bufs=1))
    persist = ctx.enter_context(tc.tile_pool(name="persist", bufs=1))
    spool = ctx.enter_context(tc.tile_pool(name="spsum", bufs=2, space="PSUM"))
    opool = ctx.enter_context(tc.tile_pool(name="opsum", bufs=4, space="PSUM"))

    # ---- constants ----
    # the big first x chunk goes onto the DMA queues before everything
    # else, split in two so both halves transfer in parallel
    xT_r = xT.rearrange("(c p) n -> p c n", p=128)
    xt0 = const.tile([128, DCH, NQ], BF16, tag="xt0")
    nc.sync.dma_start(xt0[:, 0:DCH // 2, :], xT_r[:, 0:DCH // 2, 0:NQ])
    nc.sync.dma_start(xt0[:, DCH // 2:, :], xT_r[:, DCH // 2:, 0:NQ])
    ident = const.tile([128, 128], BF16, tag="id")
    make_identity(nc, ident[:])
    w_sb = {}
    for name, ap in (("q", wq), ("k", wk), ("v", wv)):
        t = const.tile([128, DCH, 128], BF16, tag=f"w{name}")
        nc.sync.dma_start(t[:], ap.rearrange("(c p) m -> p c m", p=128))
        w_sb[name] = t


    # ---- persistent activations ----
    QT = persist.tile([128, TOK], BF16, tag="QT")
    KT = persist.tile([128, TOK], BF16, tag="KT")
    Vb = persist.tile([128, TOK // 128, 130], BF16, tag="Vb")
    OT = [persist.tile([65, TOK], BF16, tag=f"OT{h}", name=f"OT{h}")
          for h in range(2)]
    rec = [persist.tile([65, TOK], BF16, tag=f"rec{h}", name=f"rec{h}")
           for h in range(2)]
    # f32 copies of the denominator rows (partition 64) for the tail's
    # bit-trick reciprocal, and its magic constant
    drowf = persist.tile([65, 2, TOK], F32, tag="drowf")
    magic = persist.tile([65, NQ], mybir.dt.int32, tag="magic")
    nc.vector.memset(magic[:], 0x7EF311C3)

    # ---- phases 1-2: QKV projections + V transpose (scoped pools) ----
    with tc.tile_pool(name="xin", bufs=3) as xpool, \
         tc.tile_pool(name="vtp", bufs=1) as vtpool:
        VT = vtpool.tile([128, TOK], BF16, tag="VT")
        xts = {0: xt0}

        b_sb = {}
        for name, ap in (("q", bq), ("k", bk), ("v", bv)):
            t = const.tile([128, 1], F32, tag=f"b{name}")
            nc.sync.dma_start(t[:], ap[:])
            b_sb[name] = t
        # wo split into the two 64-row halves so both out-proj matmuls run
        # at partition base 0.
        wo0 = const.tile([64, D], BF16, tag="wo0")
        wo1 = const.tile([64, D], BF16, tag="wo1")
        nc.sync.dma_start(wo0[:], wo[0:64, :])
        nc.sync.dma_start(wo1[:], wo[64:128, :])
        # ones row at partition 64 for broadcasting the softmax reciprocal
        # (must share the base partition of the OT denominator row)
        ones64 = const.tile([65, 64], BF16, tag="ones64")
        nc.vector.memset(ones64[:], 1.0)
        # ones columns of V_ext (positions 64 and 129 of each 130-stripe)
        nc.vector.memset(
            Vb[:].rearrange("p t (g x) -> p t g x", g=2)[:, :, :, 64:65], 1.0)

        for t in range(TOK // NQ):
            if t in xts:
                xt = xts[t]
            else:
                xt = xpool.tile([128, DCH, NQ], BF16, tag="x")
                nc.sync.dma_start(xt[:], xT_r[:, :, t * NQ:(t + 1) * NQ])
            for name, dst in (("q", QT), ("k", KT), ("v", VT)):
                ps = opool.tile([128, NQ], F32, tag="o", name="ps_proj")
                for d in range(DCH):
                    nc.tensor.matmul(ps[:], w_sb[name][:, d, :], xt[:, d, :],
                                     start=(d == 0), stop=(d == DCH - 1))
                nc.vector.tensor_scalar_add(
                    dst[:, t * NQ:(t + 1) * NQ], ps[:], b_sb[name][:])
            # transpose this chunk's V into (tok, hd) layout right away
            # instead of a separate PE-dense phase at the end
            for u in range(4 * t, 4 * t + 4):
                pt = opool.tile([128, 128], BF16, tag="o", name="pt_tr")
                nc.tensor.transpose(pt[:], VT[:, u * 128:(u + 1) * 128],
                                    ident[:])
                nc.scalar.copy(
                    Vb[:, u, :].rearrange("p (g x) -> p g x", g=2)[:, :, 0:64],
                    pt[:].rearrange("p (g x) -> p g x", g=2))

    # ---- phase 3: attention ----
    bpool = ctx.enter_context(tc.tile_pool(name="bin", bufs=3))
    rpool = ctx.enter_context(tc.tile_pool(name="rrow", bufs=6))
    B_r = [Bh[h].rearrange("(j p) q -> p j q", p=128) for h in range(2)]

    def emit_outproj(u):
        # one 128-token chunk of the output projection + bf16 drain + DMA
        ps = spool.tile([128, 2 * NQ], F32, tag="s")
        for half in range(2):
            osl = slice(half * NQ, (half + 1) * NQ)
            nc.tensor.matmul(ps[:, osl],
                             OT[0][0:64, u * 128:(u + 1) * 128],
                             wo0[:, osl], start=True, stop=False)
            nc.tensor.matmul(ps[:, osl],
                             OT[1][0:64, u * 128:(u + 1) * 128],
                             wo1[:, osl], start=False, stop=True)
        osb = ppool.tile([128, 2 * NQ], BF16, tag="osb")
        # halves drain on ACT and DVE in parallel, each DMA'd as it lands
        nc.scalar.copy(osb[:, 0:NQ], ps[:, 0:NQ])
        nc.sync.dma_start(out[u * 128:(u + 1) * 128, 0:NQ], osb[:, 0:NQ])
        nc.vector.tensor_copy(osb[:, NQ:], ps[:, NQ:])
        nc.sync.dma_start(out[u * 128:(u + 1) * 128, NQ:], osb[:, NQ:])

    for qc in range(NQC):
        O_ps = [[opool.tile([65, NQ], F32, tag="o", name=f"O_ps{hh}{bb}")
                 for bb in range(2)] for hh in range(2)]
        for jq in range(4):  # quarter-stripes of 4 k-chunks
            # both heads interleaved per k-chunk -> one wide DVE mul later
            bstr = bpool.tile([128, 4, 2, NQ], BF16, tag="b")
            for hh in range(2):
                nc.sync.dma_start(
                    bstr[:, :, hh, :],
                    B_r[hh][:, jq * 4:(jq + 1) * 4, qc * NQ:(qc + 1) * NQ])
            for b in range(2):
                q0 = b * S + qc * NQ
                for ji in range(4):
                    j = jq * 4 + ji
                    k0 = b * S + j * 128
                    # head-packed scores: h0 in PE rows 0-63, h1 in rows
                    # 64-127, issued adjacently
                    sps = spool.tile([128, 2 * NQ], F32, tag="s")
                    for hh in range(2):
                        nc.tensor.matmul(
                            sps[:, hh * NQ:(hh + 1) * NQ],
                            KT[64 * hh:64 * hh + 64, k0:k0 + 128],
                            QT[64 * hh:64 * hh + 64, q0:q0 + NQ],
                            start=True, stop=True)
                    pe = ppool.tile([128, 2 * NQ], BF16, tag="p")
                    nc.scalar.activation(pe[:], sps[:], AF.Exp)
                    # lag bias in exp space: all-bf16 SBUF multiply on DVE
                    nc.vector.tensor_mul(
                        pe[:], pe[:],
                        bstr[:, ji, :, :].rearrange("p g q -> p (g q)"))
                    for hh in range(2):
                        nc.tensor.matmul(
                            O_ps[hh][b][:],
                            Vb[:, b * NJ + j, 65 * hh:65 * hh + 65],
                            pe[:, hh * NQ:(hh + 1) * NQ],
                            start=(j == 0), stop=(j == NJ - 1))
        for hh in range(2):
            for b in range(2):
                q0 = b * S + qc * NQ
                # stash unnormalized O^T + denominator row; normalize later.
                # Split across ACT/DVE: the next qc's first PV waits on these,
                # and ACT idles at the qc boundary anyway.
                eng = nc.scalar.copy if b == 0 else nc.vector.tensor_copy
                eng(OT[hh][:, q0:q0 + NQ], O_ps[hh][b][:])
                # f32 copy of the denominator row for the tail's reciprocal
                nc.vector.tensor_copy(drowf[64:65, hh, q0:q0 + NQ],
                                      O_ps[hh][b][64:65, :])

    # ---- phase 3b/4: recip + normalize + output projection, pipelined ----
    # -1/d rows: Ln/Exp table seed on the Scalar engine + one f32 Newton
    # step on DVE. The sign is folded into wo on the host. [1, N] rows are
    # cheap on ACT (cost scales with free size only), unlike DVE's serial
    # RECIPROCAL. Narrow chains for the first q-chunk unblock the PE fast;
    # wide chains for the remaining three quarters amortize the overhead.
    # -1/d per row via the fast-inverse bit trick (magic - bits(d)) plus one
    # f32 Newton step, entirely on DVE: ~2.5e-3 max rel error, no activation
    # tables, ~1.7us per row slice. The sign is folded into wo on the host.
    I32 = mybir.dt.int32
    rppool = ctx.enter_context(tc.tile_pool(name="rrow", bufs=6))
    for qq in range(NQC):
        for b in range(2):
            c = b * 4 + qq
            sl = slice(c * NQ, (c + 1) * NQ)
            for h in range(2):
                dr = drowf[64:65, h, sl]
                r0 = rppool.tile([65, NQ], F32, tag="r", name="r0")
                tt = rppool.tile([65, NQ], F32, tag="r", name="tt")
                nc.vector.tensor_sub(r0[64:65, :].bitcast(I32),
                                     magic[64:65, :], dr.bitcast(I32))
                nc.vector.tensor_mul(tt[64:65, :], dr, r0[64:65, :])
                with nc.allow_low_precision(reason="recip rounds to bf16"):
                    nc.vector.scalar_tensor_tensor(
                        rec[h][64:65, sl], tt[64:65, :], 2.0, r0[64:65, :],
                        mybir.AluOpType.subtract, mybir.AluOpType.mult)
                R_ps = opool.tile([64, NQ], F32, tag="o", name="R_ps")
                nc.tensor.matmul(R_ps[:], ones64[64:65, :], rec[h][64:65, sl],
                                 start=True, stop=True)
                nc.vector.tensor_mul(OT[h][0:64, sl], OT[h][0:64, sl],
                                     R_ps[:])
            for u in range(4 * c, 4 * c + 4):
                emit_outproj(u)


def build_program():
    nc = bacc.Bacc("TRN2", target_bir_lowering=False, debug=False,
                   enable_asserts=False, num_devices=N_CORES)
    aps = {}
    specs = [
        ("xT", (D, TOK), BF16), ("wq", (D, 128), BF16), ("wk", (D, 128), BF16),
        ("wv", (D, 128), BF16), ("bq", (128, 1), F32), ("bk", (128, 1), F32),
        ("bv", (128, 1), F32), ("wo", (128, D), BF16), ("B0", (S, S), BF16),
        ("B1", (S, S), BF16),
    ]
    for name, shape, dt in specs:
        aps[name] = nc.dram_tensor(name, shape, dt, kind="ExternalInput").ap()
    aps["out"] = nc.dram_tensor("out", (TOK, D), BF16,
                                kind="ExternalOutput").ap()

    with tile.TileContext(nc) as tc:
        with ExitStack() as ctx:
            _body(ctx, tc, aps)
    nc.compile()
    return nc


def _get_nc():
    global _CACHED_NC
    if _CACHED_NC is None:
        _CACHED_NC = build_program()
    return _CACHED_NC


def _host_prep(x, lag, wq, bq, wk, bk, wv, bv, wo, bo, lag_bias):
    bf16 = ml_dtypes.bfloat16
    x = np.asarray(x, dtype=np.float32)
    lag = np.asarray(lag).astype(np.int64)
    xT = np.ascontiguousarray(x.reshape(TOK, D).T.astype(bf16))
    ld = np.abs(lag[:, None] - lag[None, :]).astype(np.int64)
    lag_bias = np.asarray(lag_bias, dtype=np.float32)
    eb = np.exp(lag_bias).astype(bf16)  # (H, MAX_LAG+1) exp-space bias
    scale = np.float32(1.0 / np.sqrt(DK))
    wq = np.asarray(wq, dtype=np.float32) * scale
    bq = np.asarray(bq, dtype=np.float32) * scale
    in_maps = []
    for c in range(N_CORES):
        sl = slice(c * 128, (c + 1) * 128)
        in_maps.append({
            "xT": xT,
            "wq": np.ascontiguousarray(wq[:, sl].astype(bf16)),
            "wk": np.ascontiguousarray(np.asarray(wk, np.float32)[:, sl].astype(bf16)),
            "wv": np.ascontiguousarray(np.asarray(wv, np.float32)[:, sl].astype(bf16)),
            "bq": np.ascontiguousarray(bq[sl].reshape(128, 1)),
            "bk": np.ascontiguousarray(np.asarray(bk, np.float32)[sl].reshape(128, 1)),
            "bv": np.ascontiguousarray(np.asarray(bv, np.float32)[sl].reshape(128, 1)),
            # negated: the device's Newton recip produces -1/d, and
            # (-OT/d) @ (-wo) cancels the sign for free
            "wo": np.ascontiguousarray((-np.asarray(wo, np.float32))[sl, :].astype(bf16)),
            "B0": np.ascontiguousarray(eb[2 * c][ld]),
            "B1": np.ascontiguousarray(eb[2 * c + 1][ld]),
        })
    return in_maps


def kernel(x, lag, wq, bq, wk, bk, wv, bv, wo, bo, lag_bias):
    nc = _get_nc()
    in_maps = _host_prep(x, lag, wq, bq, wk, bk, wv, bv, wo, bo, lag_bias)
    kwargs = {}
    if TRACE:
        kwargs = dict(trace=True, tmpdir=TRACE_DIR)
    res = run_bass_kernel_spmd(nc, in_maps, core_ids=list(range(N_CORES)),
                               **kwargs)
    if TRACE:
        print(f"HW exec time: {res.exec_time_ns} ns")
    total = res.results[0]["out"].astype(np.float32)
    for c in range(1, N_CORES):
        total += res.results[c]["out"].astype(np.float32)
    total += np.asarray(bo, dtype=np.float32)[None, :]
    return total.reshape(B, S, D)


# revision 74
# speedup vs baseline: 1.0141x; 1.0141x over previous
"""MultiHeadSelfAttentionWithLagBias on 8 TRN2 NeuronCores.

Sharding: tensor-parallel over heads — 16 heads / 8 cores = 2 heads per
core. Each core computes QKV projections for its head slice (full x),
attention with the lag bias for its 2 heads over both batch elements,
and a partial output projection (its 128 rows of wo). Host sums the 8
partials and adds bo.

Device layout (per core):
  xT      (1024, 4096)  x transposed, tok = b*2048 + s, bf16
  QT/KT   (128, 4096)   q^T/k^T, partitions = [h0 dk(64) | h1 dk(64)]
  V       (128, 32, 130) per 128-tok chunk: [V_h0(64) | 1 | V_h1(64) | 1]
  scores  computed transposed: S^T (k on partitions, q on free) so the
          softmax denominator falls out of the PV matmul via the ones
          column, and O^T is produced in exactly the layout the output
          projection needs as its stationary operand.
  bias    streamed as EB_h = exp(bias_h) (2048, 2048) bf16, applied as
          pe = exp(scores) * EB on DVE (all-SBUF bf16 runs at the DVE
          2x/4x rate, unlike a PSUM f32 bias add).
  recip   softmax denominators gathered to a [128, 64] tile via an
          SBUF->SBUF DMA transpose so one wide RECIPROCAL covers all
          8192 of them (a [1, N] reciprocal is single-lane and slow).

Everything through the PE array is bf16 (power: sustained f32r work
trips the HW PE utilization throttle to 50%; bf16 also halves DMA).
"""

import ml_dtypes
import numpy as np
from contextlib import ExitStack

import concourse.bass as bass
import concourse.bacc as bacc
import concourse.mybir as mybir
import concourse.tile as tile
from concourse.bass_utils import run_bass_kernel_spmd
from concourse.masks import make_identity

F32 = mybir.dt.float32
BF16 = mybir.dt.bfloat16
AF = mybir.ActivationFunctionType

N_CORES = 8
B, S, D = 2, 2048, 1024
H, DK = 16, 64
TOK = B * S              # 4096
NQ = 512                 # q-chunk (matmul free dim)
NQC = S // NQ            # 4 q-chunks per batch
NJ = S // 128            # 16 k-chunks per batch
DCH = D // 128           # 8 contraction chunks

# Set by test.py for profiling; harness leaves these untouched.
TRACE = False
TRACE_DIR = None

_CACHED_NC = None


def _body(ctx: ExitStack, tc, aps):
    nc = tc.nc
    xT, wq, wk, wv, bq, bk, bv, wo, B0, B1, out = (
        aps["xT"], aps["wq"], aps["wk"], aps["wv"], aps["bq"], aps["bk"],
        aps["bv"], aps["wo"], aps["B0"], aps["B1"], aps["out"])
    Bh = [B0, B1]

    const = ctx.enter_context(tc.tile_pool(name="const", bufs=1))
    persist = ctx.enter_context(tc.tile_pool(name="persist", bufs=1))
    spool = ctx.enter_context(tc.tile_pool(name="spsum", bufs=2, space="PSUM"))
    opool = ctx.enter_context(tc.tile_pool(name="opsum", bufs=4, space="PSUM"))

    # ---- constants ----
    # the big first x chunk goes onto the DMA queues before everything
    # else, split in two so both halves transfer in parallel
    xT_r = xT.rearrange("(c p) n -> p c n", p=128)
    xt0 = const.tile([128, DCH, NQ], BF16, tag="xt0")
    nc.sync.dma_start(xt0[:, 0:DCH // 2, :], xT_r[:, 0:DCH // 2, 0:NQ])
    nc.sync.dma_start(xt0[:, DCH // 2:, :], xT_r[:, DCH // 2:, 0:NQ])
    ident = const.tile([128, 128], BF16, tag="id")
    make_identity(nc, ident[:])
    w_sb = {}
    for name, ap in (("q", wq), ("k", wk), ("v", wv)):
        t = const.tile([128, DCH, 128], BF16, tag=f"w{name}")
        nc.sync.dma_start(t[:], ap.rearrange("(c p) m -> p c m", p=128))
        w_sb[name] = t


    # ---- persistent activations ----
    QT = persist.tile([128, TOK], BF16, tag="QT")
    KT = persist.tile([128, TOK], BF16, tag="KT")
    Vb = persist.tile([128, TOK // 128, 130], BF16, tag="Vb")
    OT = [persist.tile([65, TOK], BF16, tag=f"OT{h}", name=f"OT{h}")
          for h in range(2)]
    rec = [persist.tile([65, TOK], BF16, tag=f"rec{h}", name=f"rec{h}")
           for h in range(2)]
    # f32 copies of the denominator rows (partition 64) for the tail's
    # bit-trick reciprocal, and its magic constant
    drowf = persist.tile([65, 2, TOK], F32, tag="drowf")
    magic = persist.tile([65, NQ], mybir.dt.int32, tag="magic")
    nc.vector.memset(magic[:], 0x7EF311C3)

    # ---- phases 1-2: QKV projections + V transpose (scoped pools) ----
    with tc.tile_pool(name="xin", bufs=3) as xpool, \
         tc.tile_pool(name="vtp", bufs=1) as vtpool:
        VT = vtpool.tile([128, TOK], BF16, tag="VT")
        xts = {0: xt0}

        b_sb = {}
        for name, ap in (("q", bq), ("k", bk), ("v", bv)):
            t = const.tile([128, 1], F32, tag=f"b{name}")
            nc.sync.dma_start(t[:], ap[:])
            b_sb[name] = t
        # wo split into the two 64-row halves so both out-proj matmuls run
        # at partition base 0.
        wo0 = const.tile([64, D], BF16, tag="wo0")
        wo1 = const.tile([64, D], BF16, tag="wo1")
        nc.sync.dma_start(wo0[:], wo[0:64, :])
        nc.sync.dma_start(wo1[:], wo[64:128, :])
        # ones row at partition 64 for broadcasting the softmax reciprocal
        # (must share the base partition of the OT denominator row)
        ones64 = const.tile([65, 64], BF16, tag="ones64")
        nc.vector.memset(ones64[:], 1.0)
        # ones columns of V_ext (positions 64 and 129 of each 130-stripe)
        nc.vector.memset(
            Vb[:].rearrange("p t (g x) -> p t g x", g=2)[:, :, :, 64:65], 1.0)

        for t in range(TOK // NQ):
            if t in xts:
                xt = xts[t]
            else:
                xt = xpool.tile([128, DCH, NQ], BF16, tag="x")
                nc.sync.dma_start(xt[:], xT_r[:, :, t * NQ:(t + 1) * NQ])
            for name, dst in (("q", QT), ("k", KT), ("v", VT)):
                ps = opool.tile([128, NQ], F32, tag="o", name="ps_proj")
                for d in range(DCH):
                    nc.tensor.matmul(ps[:], w_sb[name][:, d, :], xt[:, d, :],
                                     start=(d == 0), stop=(d == DCH - 1))
                nc.vector.tensor_scalar_add(
                    dst[:, t * NQ:(t + 1) * NQ], ps[:], b_sb[name][:])

        # V transpose into (tok, hd) chunks
        for u in range(TOK // 128):
            pt = opool.tile([128, 128], BF16, tag="o", name="pt_tr")
            nc.tensor.transpose(pt[:], VT[:, u * 128:(u + 1) * 128], ident[:])
            nc.scalar.copy(
                Vb[:, u, :].rearrange("p (g x) -> p g x", g=2)[:, :, 0:64],
                pt[:].rearrange("p (g x) -> p g x", g=2))

    # ---- phase 3: attention ----
    bpool = ctx.enter_context(tc.tile_pool(name="bin", bufs=3))
    rpool = ctx.enter_context(tc.tile_pool(name="rrow", bufs=6))
    B_r = [Bh[h].rearrange("(j p) q -> p j q", p=128) for h in range(2)]

    def emit_outproj(u):
        # one 128-token chunk of the output projection + bf16 drain + DMA
        ps = spool.tile([128, 2 * NQ], F32, tag="s")
        for half in range(2):
            osl = slice(half * NQ, (half + 1) * NQ)
            nc.tensor.matmul(ps[:, osl],
                             OT[0][0:64, u * 128:(u + 1) * 128],
                             wo0[:, osl], start=True, stop=False)
            nc.tensor.matmul(ps[:, osl],
                             OT[1][0:64, u * 128:(u + 1) * 128],
                             wo1[:, osl], start=False, stop=True)
        osb = ppool.tile([128, 2 * NQ], BF16, tag="osb")
        # halves drain on ACT and DVE in parallel, each DMA'd as it lands
        nc.scalar.copy(osb[:, 0:NQ], ps[:, 0:NQ])
        nc.sync.dma_start(out[u * 128:(u + 1) * 128, 0:NQ], osb[:, 0:NQ])
        nc.vector.tensor_copy(osb[:, NQ:], ps[:, NQ:])
        nc.sync.dma_start(out[u * 128:(u + 1) * 128, NQ:], osb[:, NQ:])

    for qc in range(NQC):
        O_ps = [[opool.tile([65, NQ], F32, tag="o", name=f"O_ps{hh}{bb}")
                 for bb in range(2)] for hh in range(2)]
        for jq in range(4):  # quarter-stripes of 4 k-chunks
            # both heads interleaved per k-chunk -> one wide DVE mul later
            bstr = bpool.tile([128, 4, 2, NQ], BF16, tag="b")
            for hh in range(2):
                nc.sync.dma_start(
                    bstr[:, :, hh, :],
                    B_r[hh][:, jq * 4:(jq + 1) * 4, qc * NQ:(qc + 1) * NQ])
            for b in range(2):
                q0 = b * S + qc * NQ
                for ji in range(4):
                    j = jq * 4 + ji
                    k0 = b * S + j * 128
                    # head-packed scores: h0 in PE rows 0-63, h1 in rows
                    # 64-127, issued adjacently
                    sps = spool.tile([128, 2 * NQ], F32, tag="s")
                    for hh in range(2):
                        nc.tensor.matmul(
                            sps[:, hh * NQ:(hh + 1) * NQ],
                            KT[64 * hh:64 * hh + 64, k0:k0 + 128],
                            QT[64 * hh:64 * hh + 64, q0:q0 + NQ],
                            start=True, stop=True)
                    pe = ppool.tile([128, 2 * NQ], BF16, tag="p")
                    nc.scalar.activation(pe[:], sps[:], AF.Exp)
                    # lag bias in exp space: all-bf16 SBUF multiply on DVE
                    nc.vector.tensor_mul(
                        pe[:], pe[:],
                        bstr[:, ji, :, :].rearrange("p g q -> p (g q)"))
                    for hh in range(2):
                        nc.tensor.matmul(
                            O_ps[hh][b][:],
                            Vb[:, b * NJ + j, 65 * hh:65 * hh + 65],
                            pe[:, hh * NQ:(hh + 1) * NQ],
                            start=(j == 0), stop=(j == NJ - 1))
        for hh in range(2):
            for b in range(2):
                q0 = b * S + qc * NQ
                # stash unnormalized O^T + denominator row; normalize later.
                # Split across ACT/DVE: the next qc's first PV waits on these,
                # and ACT idles at the qc boundary anyway.
                eng = nc.scalar.copy if b == 0 else nc.vector.tensor_copy
                eng(OT[hh][:, q0:q0 + NQ], O_ps[hh][b][:])
                # f32 copy of the denominator row for the tail's reciprocal
                nc.vector.tensor_copy(drowf[64:65, hh, q0:q0 + NQ],
                                      O_ps[hh][b][64:65, :])

    # ---- phase 3b/4: recip + normalize + output projection, pipelined ----
    # -1/d rows: Ln/Exp table seed on the Scalar engine + one f32 Newton
    # step on DVE. The sign is folded into wo on the host. [1, N] rows are
    # cheap on ACT (cost scales with free size only), unlike DVE's serial
    # RECIPROCAL. Narrow chains for the first q-chunk unblock the PE fast;
    # wide chains for the remaining three quarters amortize the overhead.
    # -1/d per row via the fast-inverse bit trick (magic - bits(d)) plus one
    # f32 Newton step, entirely on DVE: ~2.5e-3 max rel error, no activation
    # tables, ~1.7us per row slice. The sign is folded into wo on the host.
    I32 = mybir.dt.int32
    rppool = ctx.enter_context(tc.tile_pool(name="rrow", bufs=6))
    for qq in range(NQC):
        for b in range(2):
            c = b * 4 + qq
            sl = slice(c * NQ, (c + 1) * NQ)
            for h in range(2):
                dr = drowf[64:65, h, sl]
                r0 = rppool.tile([65, NQ], F32, tag="r", name="r0")
                tt = rppool.tile([65, NQ], F32, tag="r", name="tt")
                nc.vector.tensor_sub(r0[64:65, :].bitcast(I32),
                                     magic[64:65, :], dr.bitcast(I32))
                nc.vector.tensor_mul(tt[64:65, :], dr, r0[64:65, :])
                with nc.allow_low_precision(reason="recip rounds to bf16"):
                    nc.vector.scalar_tensor_tensor(
                        rec[h][64:65, sl], tt[64:65, :], 2.0, r0[64:65, :],
                        mybir.AluOpType.subtract, mybir.AluOpType.mult)
                R_ps = opool.tile([64, NQ], F32, tag="o", name="R_ps")
                nc.tensor.matmul(R_ps[:], ones64[64:65, :], rec[h][64:65, sl],
                                 start=True, stop=True)
                nc.vector.tensor_mul(OT[h][0:64, sl], OT[h][0:64, sl],
                                     R_ps[:])
            for u in range(4 * c, 4 * c + 4):
                emit_outproj(u)


def build_program():
    nc = bacc.Bacc("TRN2", target_bir_lowering=False, debug=False,
                   enable_asserts=False, num_devices=N_CORES)
    aps = {}
    specs = [
        ("xT", (D, TOK), BF16), ("wq", (D, 128), BF16), ("wk", (D, 128), BF16),
        ("wv", (D, 128), BF16), ("bq", (128, 1), F32), ("bk", (128, 1), F32),
        ("bv", (128, 1), F32), ("wo", (128, D), BF16), ("B0", (S, S), BF16),
        ("B1", (S, S), BF16),
    ]
    for name, shape, dt in specs:
        aps[name] = nc.dram_tensor(name, shape, dt, kind="ExternalInput").ap()
    aps["out"] = nc.dram_tensor("out", (TOK, D), BF16,
                                kind="ExternalOutput").ap()

    with tile.TileContext(nc) as tc:
        with ExitStack() as ctx:
            _body(ctx, tc, aps)
    nc.compile()
    return nc


def _get_nc():
    global _CACHED_NC
    if _CACHED_NC is None:
        _CACHED_NC = build_program()
    return _CACHED_NC


def _host_prep(x, lag, wq, bq, wk, bk, wv, bv, wo, bo, lag_bias):
    bf16 = ml_dtypes.bfloat16
    x = np.asarray(x, dtype=np.float32)
    lag = np.asarray(lag).astype(np.int64)
    xT = np.ascontiguousarray(x.reshape(TOK, D).T.astype(bf16))
    ld = np.abs(lag[:, None] - lag[None, :]).astype(np.int64)
    lag_bias = np.asarray(lag_bias, dtype=np.float32)
    eb = np.exp(lag_bias).astype(bf16)  # (H, MAX_LAG+1) exp-space bias
    scale = np.float32(1.0 / np.sqrt(DK))
    wq = np.asarray(wq, dtype=np.float32) * scale
    bq = np.asarray(bq, dtype=np.float32) * scale
    in_maps = []
    for c in range(N_CORES):
        sl = slice(c * 128, (c + 1) * 128)
        in_maps.append({
            "xT": xT,
            "wq": np.ascontiguousarray(wq[:, sl].astype(bf16)),
            "wk": np.ascontiguousarray(np.asarray(wk, np.float32)[:, sl].astype(bf16)),
            "wv": np.ascontiguousarray(np.asarray(wv, np.float32)[:, sl].astype(bf16)),
            "bq": np.ascontiguousarray(bq[sl].reshape(128, 1)),
            "bk": np.ascontiguousarray(np.asarray(bk, np.float32)[sl].reshape(128, 1)),
            "bv": np.ascontiguousarray(np.asarray(bv, np.float32)[sl].reshape(128, 1)),
            # negated: the device's Newton recip produces -1/d, and
            # (-OT/d) @ (-wo) cancels the sign for free
            "wo": np.ascontiguousarray((-np.asarray(wo, np.float32))[sl, :].astype(bf16)),
            "B0": np.ascontiguousarray(eb[2 * c][ld]),
            "B1": np.ascontiguousarray(eb[2 * c + 1][ld]),
        })
    return in_maps


def kernel(x, lag, wq, bq, wk, bk, wv, bv, wo, bo, lag_bias):
    nc = _get_nc()
    in_maps = _host_prep(x, lag, wq, bq, wk, bk, wv, bv, wo, bo, lag_bias)
    kwargs = {}
    if TRACE:
        kwargs = dict(trace=True, tmpdir=TRACE_DIR)
    res = run_bass_kernel_spmd(nc, in_maps, core_ids=list(range(N_CORES)),
                               **kwargs)
    if TRACE:
        print(f"HW exec time: {res.exec_time_ns} ns")
    total = res.results[0]["out"].astype(np.float32)
    for c in range(1, N_CORES):
        total += res.results[c]["out"].astype(np.float32)
    total += np.asarray(bo, dtype=np.float32)[None, :]
    return total.reshape(B, S, D)
